# revision 10
# baseline (speedup 1.0000x reference)
"""v3: software-pipelined variant of kernel.py (see kernel.py docstring).

Projections (PE-dense) are emitted interleaved with attention groups
(ACT-bound) so the PE fills attention's exp-wait bubbles with the next
batch/rowtile's projection matmuls. Stage layout:

  PRE: A(b0, rt0-1)
  (b0,j0)xA(b0,rt2-3) (b0,j1)xA(rt4-5) (b0,j2)xA(rt6-7) (b0,j3)xA(b1,rt0-1)
  [AG b0]
  (b1,j0)xA(b1,rt2-3) ... (b1,j3)x[k-cache transposes]
  [AG b1]
  C: kv-cache DMA + output projection (AG-dependent)
"""

import sys

if "/opt/trn_rl_repo" not in sys.path:
    sys.path.insert(0, "/opt/trn_rl_repo")

import numpy as np

import concourse.bass as bass
import concourse.mybir as mybir
import concourse.tile as tile
from concourse import bacc, bass_utils
from concourse.masks import make_identity

F32 = mybir.dt.float32
F32R = mybir.dt.float32r
EXP = mybir.ActivationFunctionType.Exp
COPY = mybir.ActivationFunctionType.Copy

B, S, D = 2, 2048, 2048
H, KVH, HD = 32, 8, 64
GROUPS = H // KVH
QH = GROUPS
NCORES = 8
KT = D // 128
RT = 256
NRT = S // RT
QT = 512
NQT = S // NQT if False else S // QT
NKEYT = S // 128


def _build(collective=True):
    nc = bacc.Bacc("TRN2", target_bir_lowering=False, debug=False, num_devices=NCORES)

    xT = nc.dram_tensor("xT", [B, D, S], F32R, kind="ExternalInput").ap()
    wqT = nc.dram_tensor("wqT", [D, QH * HD], F32R, kind="ExternalInput").ap()
    wkT = nc.dram_tensor("wkT", [D, HD], F32R, kind="ExternalInput").ap()
    wvT = nc.dram_tensor("wvT", [D, HD], F32R, kind="ExternalInput").ap()
    woT = nc.dram_tensor("woT", [D, D], F32R, kind="ExternalInput").ap()
    maskTd = nc.dram_tensor("maskTd", [128, 128], F32, kind="ExternalInput").ap()
    mstepA = nc.dram_tensor(
        "mstepA", [128, 128], mybir.dt.bfloat16, kind="ExternalInput"
    ).ap()
    mstepB = nc.dram_tensor(
        "mstepB", [128, 128], mybir.dt.bfloat16, kind="ExternalInput"
    ).ap()

    y = nc.dram_tensor("y", [QT, D], F32, kind="ExternalOutput").ap()
    k_out = nc.dram_tensor("k_out", [B, S, HD], F32, kind="ExternalOutput").ap()
    v_out = nc.dram_tensor("v_out", [B, S, HD], F32, kind="ExternalOutput").ap()

    with tile.TileContext(nc, num_cores=NCORES) as tc, nc.allow_low_precision(
        reason="fp32r compute throughout"
    ):
        with (
            tc.tile_pool(name="consts", bufs=1) as consts,
            tc.tile_pool(name="qkv", bufs=1) as qkv,
            tc.tile_pool(name="dram", bufs=1, space="DRAM") as dram,
        ):
            ident_f = consts.tile([128, 128], F32)
            make_identity(nc, ident_f[:])
            ident = consts.tile([128, 128], F32R)
            nc.vector.tensor_copy(ident[:], ident_f[:])
            maskT = consts.tile([128, 128], F32)
            nc.sync.dma_start(out=maskT[:], in_=maskTd)
            mstepA_sb = consts.tile([128, 128], mybir.dt.bfloat16)
            nc.sync.dma_start(out=mstepA_sb[:], in_=mstepA)
            mstepB_sb = consts.tile([128, 128], mybir.dt.bfloat16)
            nc.sync.dma_start(out=mstepB_sb[:], in_=mstepB)
            ones_f = consts.tile([128, 64], F32)
            nc.vector.memset(ones_f[:], 1.0)
            ones_r = consts.tile([128, 64], F32R)
            nc.vector.tensor_copy(ones_r[:], ones_f[:])

            qT = [
                [qkv.tile([128, S], F32R, name=f"qT_b{b}_h{h}") for h in range(QH)]
                for b in range(B)
            ]
            kT = [qkv.tile([128, S], F32R, name=f"kT_b{b}") for b in range(B)]
            vaug = qkv.tile([128, B * NKEYT * 65], F32R)

            ag_in = dram.tile([B, QH * HD, S], F32R)
            ag_out = dram.tile([B, NCORES * QH * HD, S], F32R)

            with (
                tc.tile_pool(name="wpool", bufs=1) as wpool,
                tc.tile_pool(name="xpool", bufs=2) as xpool,
                tc.tile_pool(name="stageA", bufs=3) as stageA,
                tc.tile_pool(name="stageB", bufs=3) as stageB,
                tc.tile_pool(name="attnp", bufs=3) as attnp,
                tc.tile_pool(name="kcache", bufs=1) as kcache,
                tc.tile_pool(name="psA", bufs=2, space="PSUM") as psA,
                tc.tile_pool(name="psP", bufs=2, space="PSUM") as psP,
                tc.tile_pool(name="psO", bufs=2, space="PSUM") as psO,
            ):
                k_sb = kcache.tile([128, B * NKEYT * 64], F32)

                wk_sb = wpool.tile([128, KT * HD], F32R)
                nc.sync.dma_start(
                    out=wk_sb[:].rearrange("p (kt m) -> p kt m", kt=KT),
                    in_=wkT.rearrange("(kt p) m -> p kt m", p=128),
                )
                wv_sb = wpool.tile([128, KT * HD], F32R)
                nc.sync.dma_start(
                    out=wv_sb[:].rearrange("p (kt m) -> p kt m", kt=KT),
                    in_=wvT.rearrange("(kt p) m -> p kt m", p=128),
                )
                wq_sb = wpool.tile([128, KT * QH * HD], F32R)
                for qa in range(4):
                    nc.sync.dma_start(
                        out=wq_sb[:].rearrange("p (kt m) -> p kt m", kt=KT)[
                            :, qa * 4 : qa * 4 + 4
                        ],
                        in_=wqT.rearrange("(kt p) m -> p kt m", p=128)[
                            :, qa * 4 : qa * 4 + 4
                        ],
                    )

                # ---------- A-unit generator: yields after each subunit ----------
                def gen_A():
                    for b in range(B):
                        for rt in range(NRT):
                            r0 = rt * RT
                            x_t = xpool.tile([128, KT * RT], F32R, name="x_t")
                            for qa in range(4):
                                nc.scalar.dma_start(
                                    out=x_t[:].rearrange(
                                        "p (kt r) -> p kt r", kt=KT
                                    )[:, qa * 4 : qa * 4 + 4],
                                    in_=xT[b].rearrange(
                                        "(kt p) r -> p kt r", p=128
                                    )[:, qa * 4 : qa * 4 + 4, r0 : r0 + RT],
                                )
                            ps_k = psA.tile([64, RT], F32, name="ps_k", tag="psA")
                            for kt in range(KT):
                                nc.tensor.matmul(
                                    ps_k[:],
                                    wk_sb[:, kt * HD : (kt + 1) * HD],
                                    x_t[:, kt * RT : (kt + 1) * RT],
                                    start=(kt == 0),
                                    stop=(kt == KT - 1),
                                )
                            nc.vector.tensor_copy(kT[b][0:64, r0 : r0 + RT], ps_k[:])
                            yield
                            for m in range(2):
                                ps_q = psA.tile([128, RT], F32, name="ps_q", tag="psA")
                                for kt in range(KT):
                                    nc.tensor.matmul(
                                        ps_q[:],
                                        wq_sb[
                                            :,
                                            kt * 256
                                            + m * 128 : kt * 256
                                            + m * 128
                                            + 128,
                                        ],
                                        x_t[:, kt * RT : (kt + 1) * RT],
                                        start=(kt == 0),
                                        stop=(kt == KT - 1),
                                    )
                                nc.vector.tensor_copy(
                                    qT[b][2 * m][0:64, r0 : r0 + RT], ps_q[0:64, :]
                                )
                                nc.vector.tensor_copy(
                                    qT[b][2 * m + 1][64:128, r0 : r0 + RT],
                                    ps_q[64:128, :],
                                )
                                yield
                            ps_v = psA.tile([64, RT], F32, name="ps_v", tag="psA")
                            for kt in range(KT):
                                nc.tensor.matmul(
                                    ps_v[:],
                                    wv_sb[:, kt * HD : (kt + 1) * HD],
                                    x_t[:, kt * RT : (kt + 1) * RT],
                                    start=(kt == 0),
                                    stop=(kt == KT - 1),
                                )
                            vT_st = stageA.tile([64, RT], F32R, name="vT_st")
                            nc.vector.tensor_copy(vT_st[:], ps_v[:])
                            for sub in range(RT // 128):
                                keyt = (r0 + sub * 128) // 128
                                ps_t = psA.tile(
                                    [128, 64], F32R, name="ps_t", tag="psA"
                                )
                                nc.tensor.matmul(
                                    ps_t[:],
                                    vT_st[:, sub * 128 : (sub + 1) * 128],
                                    ident[0:64, 0:64],
                                    is_transpose=True,
                                )
                                col = (b * NKEYT + keyt) * 65
                                nc.vector.tensor_copy(
                                    vaug[:, col : col + 64], ps_t[:]
                                )
                                nc.vector.tensor_copy(
                                    vaug[:, col + 64 : col + 65], ones_r[:, 0:1]
                                )
                            if rt % 2 == 1:
                                # pair complete: dup kv/q dims to other half
                                p0 = (rt - 1) * RT
                                nc.sync.dma_start(
                                    out=kT[b][64:128, p0 : p0 + 2 * RT],
                                    in_=kT[b][0:64, p0 : p0 + 2 * RT],
                                )
                                for h in range(QH):
                                    if h % 2 == 0:
                                        nc.sync.dma_start(
                                            out=qT[b][h][64:128, p0 : p0 + 2 * RT],
                                            in_=qT[b][h][0:64, p0 : p0 + 2 * RT],
                                        )
                                    else:
                                        nc.sync.dma_start(
                                            out=qT[b][h][0:64, p0 : p0 + 2 * RT],
                                            in_=qT[b][h][64:128, p0 : p0 + 2 * RT],
                                        )
                            yield
                    # tail filler: k-cache transposes
                    for b in range(B):
                        for pair in range(NKEYT // 4):
                            for sub in range(4):
                                keyt = pair * 4 + sub
                                ps_t2 = psA.tile(
                                    [128, 64], F32R, name="ps_t2", tag="psA"
                                )
                                nc.tensor.matmul(
                                    ps_t2[:],
                                    kT[b][0:64, keyt * 128 : (keyt + 1) * 128],
                                    ident[0:64, 0:64],
                                    is_transpose=True,
                                )
                                colk = (b * NKEYT + keyt) * 64
                                nc.vector.tensor_copy(
                                    k_sb[:, colk : colk + 64], ps_t2[:]
                                )
                            yield
                    while True:
                        yield  # exhausted: no-op filler

                gA = gen_A()

                def take_A(n):
                    for _ in range(n):
                        next(gA)

                # ---------- B stage emitter ----------
                def emit_B_stage(b, j, n_fill):
                    nond = list(range(4 * j))
                    groups = [
                        [(nond[i], 0, QT), (nond[i + 1], 0, QT)]
                        for i in range(0, len(nond), 2)
                    ]
                    d0 = 4 * j
                    groups.append([(d0, 0, 512), (d0 + 1, 128, 384)])
                    groups.append([(d0 + 2, 256, 256), (d0 + 3, 384, 128)])
                    last_kt = 4 * j + 3
                    total_groups = len(groups) * QH
                    gi = 0
                    filled = 0
                    for h in range(QH):
                        ps_o = psO.tile([65, QT], F32, name="ps_o", tag="psO")
                        for grp in groups:
                            width = sum(g[2] for g in grp)
                            ps_p = psP.tile([128, 1024], F32, name="ps_p", tag="psP")
                            col = 0
                            for pos, (kt, qoff, n_q) in enumerate(grp):
                                p0 = 64 * (pos % 2)
                                diag = kt >= d0
                                if diag:
                                    # prefill the causal corner mask into PSUM
                                    # (bank-clearing start=True), then let the
                                    # scores matmul accumulate on top.
                                    nc.tensor.matmul(
                                        ps_p[:, col : col + 128],
                                        mstepA_sb[:],
                                        mstepB_sb[:],
                                    )
                                nc.tensor.matmul(
                                    ps_p[:, col : col + n_q],
                                    kT[b][p0 : p0 + 64, kt * 128 : (kt + 1) * 128],
                                    qT[b][h][
                                        p0 : p0 + 64, j * QT + qoff : (j + 1) * QT
                                    ],
                                    start=not diag,
                                    stop=True,
                                    skip_group_check=True,
                                )
                                col += n_q
                            at = attnp.tile([128, 1024], F32R, name="at")
                            nc.scalar.activation(
                                at[:, 0:width], ps_p[:, 0:width], EXP, scale=0.125
                            )
                            col = 0
                            for kt, qoff, n_q in grp:
                                vcol = (b * NKEYT + kt) * 65
                                nc.tensor.matmul(
                                    ps_o[:, qoff:QT],
                                    vaug[:, vcol : vcol + 65],
                                    at[:, col : col + n_q],
                                    start=(kt == 0),
                                    stop=(kt == last_kt),
                                )
                                col += n_q
                            gi += 1
                            want = gi * n_fill // total_groups
                            # clump fills into bursts of >=3 subunits so the
                            # PE gets ~5us dense stretches (HAM warm-up).
                            if want - filled >= 3 or (want > filled and gi == total_groups):
                                take_A(want - filled)
                                filled = want
                        recip = stageB.tile([65, QT], F32R, name="recip")
                        nc.vector.reciprocal(recip[64:65, :], ps_o[64:65, :])
                        bc_ps = psA.tile([64, QT], F32, name="bc_ps", tag="psA")
                        nc.tensor.matmul(bc_ps[:], ones_r[64:65, :], recip[64:65, :])
                        bc_sb = stageB.tile([64, QT], F32, name="bc_sb")
                        nc.scalar.activation(bc_sb[:], bc_ps[:], COPY)
                        outT_t = stageB.tile([64, QT], F32R, name="outT_t")
                        nc.vector.tensor_mul(outT_t[:], ps_o[0:64, :], bc_sb[:])
                        nc.sync.dma_start(
                            out=ag_in[b][
                                h * HD : (h + 1) * HD, j * QT : (j + 1) * QT
                            ],
                            in_=outT_t[:],
                        )
                    if filled < n_fill:
                        take_A(n_fill - filled)

                # ---------- pipeline ----------
                take_A(8)  # PRE: b0 rt0-1
                for b in range(B):
                    for j in range(NQT):
                        emit_B_stage(b, j, 8)
                        if b == 0 and j == 2:
                            nc.gpsimd.dma_start(
                                out=v_out[0].rearrange("(kt p) d -> p kt d", p=128),
                                in_=vaug[:, 0 : NKEYT * 65]
                                .rearrange("p (kt e) -> p kt e", e=65)[:, :, 0:64],
                            )
                    if collective:
                        nc.gpsimd.collective_compute(
                            "AllGather",
                            mybir.AluOpType.bypass,
                            replica_groups=[list(range(NCORES))],
                            ins=[ag_in[b].opt()],
                            outs=[ag_out[b].opt()],
                        )
                    else:
                        nc.sync.dma_start(
                            out=ag_out[b][0 : QH * HD, :], in_=ag_in[b]
                        )
                nc.gpsimd.dma_start(
                    out=v_out[1].rearrange("(kt p) d -> p kt d", p=128),
                    in_=vaug[:, NKEYT * 65 : 2 * NKEYT * 65]
                    .rearrange("p (kt e) -> p kt e", e=65)[:, :, 0:64],
                )
                for b in range(B):
                    nc.sync.dma_start(
                        out=k_out[b].rearrange("(kt p) d -> p kt d", p=128),
                        in_=k_sb[
                            :, b * NKEYT * 64 : (b + 1) * NKEYT * 64
                        ].rearrange("p (kt d) -> p kt d", kt=NKEYT),
                    )

            # ========== Phase C: output projection ==========
            with (
                tc.tile_pool(name="chp", bufs=1) as chp,
                tc.tile_pool(name="wop", bufs=16) as wop,
                tc.tile_pool(name="yp", bufs=2) as yp,
                tc.tile_pool(name="psY", bufs=6, space="PSUM") as psY,
            ):
                cid = nc.sync.partition_id()
                my_b = cid // 4
                my_col = (cid % 4) * QT
                ch_all = chp.tile([128, KT * QT], F32R, name="ch_all")
                for qa in range(4):
                    nc.sync.dma_start(
                        out=ch_all[:].rearrange("p (kt c) -> p kt c", kt=KT)[
                            :, qa * 4 : qa * 4 + 4
                        ],
                        in_=ag_out.rearrange("b (kt p) c -> p kt b c", p=128)[
                            :, qa * 4 : qa * 4 + 4, bass.ds(my_b, 1), bass.ds(my_col, QT)
                        ].rearrange("p kt b c -> p kt (b c)"),
                    )
                chunks = [ch_all[:, kt2 * QT : (kt2 + 1) * QT] for kt2 in range(KT)]
                for n in range(4):
                    ps_y = [
                        psY.tile([128, 512], F32, name=f"ps_y{mm}", tag="psY")
                        for mm in range(4)
                    ]
                    for kt2 in range(KT):
                        wo_t = wop.tile([128, 512], F32R, name="wo_t")
                        nc.sync.dma_start(
                            out=wo_t[:],
                            in_=woT[
                                kt2 * 128 : (kt2 + 1) * 128, n * 512 : (n + 1) * 512
                            ],
                        )
                        for mm in range(4):
                            nc.tensor.matmul(
                                ps_y[mm][:],
                                chunks[kt2][:, mm * 128 : (mm + 1) * 128],
                                wo_t[:],
                                start=(kt2 == 0),
                                stop=(kt2 == KT - 1),
                            )
                    for mm in range(4):
                        y_sb = yp.tile([128, 512], F32, name="y_sb")
                        nc.scalar.activation(y_sb[:], ps_y[mm][:], COPY)
                        nc.sync.dma_start(
                            out=y[mm * 128 : (mm + 1) * 128, n * 512 : (n + 1) * 512],
                            in_=y_sb[:],
                        )

    nc.compile()
    return nc


_cache = {}


def _get_nc():
    if "nc" not in _cache:
        _cache["nc"] = _build(collective=True)
    return _cache["nc"]


def _numpy_reference(x, mask, Wq, Wk, Wv, Wo):
    """Fallback for non-causal masks: straight numpy implementation."""
    b, s, _ = x.shape
    q = (x @ Wq.T).reshape(b, s, H, HD).transpose(0, 2, 1, 3)
    k = (x @ Wk.T).reshape(b, s, KVH, HD).transpose(0, 2, 1, 3)
    v = (x @ Wv.T).reshape(b, s, KVH, HD).transpose(0, 2, 1, 3)
    kr = np.repeat(k, GROUPS, axis=1)
    vr = np.repeat(v, GROUPS, axis=1)
    scores = np.einsum("bhqd,bhkd->bhqk", q, kr) / np.sqrt(np.float32(HD))
    scores = scores + mask
    scores = scores - scores.max(axis=-1, keepdims=True)
    e = np.exp(scores)
    attn = e / e.sum(axis=-1, keepdims=True)
    out = np.einsum("bhqk,bhkd->bhqd", attn, vr)
    out = out.transpose(0, 2, 1, 3).reshape(b, s, H * HD)
    return (out @ Wo.T).astype(np.float32), k, v


def kernel(x, attention_mask, Wq, Wk, Wv, Wo):
    import ml_dtypes

    x = np.ascontiguousarray(np.asarray(x, dtype=np.float32))
    mask = np.ascontiguousarray(np.asarray(attention_mask, dtype=np.float32))
    Wq = np.ascontiguousarray(np.asarray(Wq, dtype=np.float32))
    Wk = np.ascontiguousarray(np.asarray(Wk, dtype=np.float32))
    Wv = np.ascontiguousarray(np.asarray(Wv, dtype=np.float32))
    Wo = np.ascontiguousarray(np.asarray(Wo, dtype=np.float32))

    # fast path requires the standard causal additive mask: zeros on/below
    # the diagonal, <= -1e8 above it.
    m2 = mask.reshape(S, S)
    causal = bool(
        (m2[np.tril_indices(S)] == 0.0).all()
        and (m2[np.triu_indices(S, k=1)] <= -1e8).all()
    )
    if not causal:
        return _numpy_reference(x, mask, Wq, Wk, Wv, Wo)

    xT = np.ascontiguousarray(x.transpose(0, 2, 1))
    woT = np.ascontiguousarray(Wo.T)
    maskTd = np.ascontiguousarray(m2[0:128, 0:128].T)
    # step matrices for the PSUM mask prefill:
    # mask[dk, q] = sum_i A[i, dk] * Bm[i, q]  (= -(dk-q)*1e9 above diagonal)
    ii = np.arange(128)
    mA = (ii[:, None] <= ii[None, :]).astype(ml_dtypes.bfloat16)  # [i, dk]
    mBm = np.where(ii[:, None] > ii[None, :], np.float32(-1e9), 0.0).astype(
        ml_dtypes.bfloat16
    )  # [i, q]
    in_maps = []
    for c in range(NCORES):
        in_maps.append(
            {
                "xT": xT,
                "wqT": np.ascontiguousarray(Wq[256 * c : 256 * (c + 1), :].T),
                "wkT": np.ascontiguousarray(Wk[64 * c : 64 * (c + 1), :].T),
                "wvT": np.ascontiguousarray(Wv[64 * c : 64 * (c + 1), :].T),
                "woT": woT,
                "maskTd": maskTd,
                "mstepA": mA,
                "mstepB": mBm,
            }
        )

    res = bass_utils.run_bass_kernel_spmd(
        _get_nc(), in_maps, core_ids=list(range(NCORES))
    )

    out = np.empty((B, S, D), np.float32)
    k = np.empty((B, KVH, S, HD), np.float32)
    v = np.empty((B, KVH, S, HD), np.float32)
    for c in range(NCORES):
        r = res.results[c]
        out[c // 4, 512 * (c % 4) : 512 * (c % 4) + 512, :] = r["y"]
        k[:, c, :, :] = r["k_out"]
        v[:, c, :, :] = r["v_out"]
    return out, k, v


# revision 11
# speedup vs baseline: 1.1488x; 1.1488x over previous
"""v3: software-pipelined variant of kernel.py (see kernel.py docstring).

Projections (PE-dense) are emitted interleaved with attention groups
(ACT-bound) so the PE fills attention's exp-wait bubbles with the next
batch/rowtile's projection matmuls. Stage layout:

  PRE: A(b0, rt0-1)
  (b0,j0)xA(b0,rt2-3) (b0,j1)xA(rt4-5) (b0,j2)xA(rt6-7) (b0,j3)xA(b1,rt0-1)
  [AG b0]
  (b1,j0)xA(b1,rt2-3) ... (b1,j3)x[k-cache transposes]
  [AG b1]
  C: kv-cache DMA + output projection (AG-dependent)
"""

import sys

if "/opt/trn_rl_repo" not in sys.path:
    sys.path.insert(0, "/opt/trn_rl_repo")

import numpy as np

import concourse.bass as bass
import concourse.mybir as mybir
import concourse.tile as tile
from concourse import bacc, bass_utils
from concourse.masks import make_identity

F32 = mybir.dt.float32
F32R = mybir.dt.float32r
EXP = mybir.ActivationFunctionType.Exp
COPY = mybir.ActivationFunctionType.Copy

B, S, D = 2, 2048, 2048
H, KVH, HD = 32, 8, 64
GROUPS = H // KVH
QH = GROUPS
NCORES = 8
KT = D // 128
RT = 256
NRT = S // RT
QT = 512
NQT = S // NQT if False else S // QT
NKEYT = S // 128


def _build(collective=True):
    nc = bacc.Bacc("TRN2", target_bir_lowering=False, debug=False, num_devices=NCORES)

    xT = nc.dram_tensor("xT", [B, D, S], F32R, kind="ExternalInput").ap()
    wqT = nc.dram_tensor("wqT", [D, QH * HD], F32R, kind="ExternalInput").ap()
    wkT = nc.dram_tensor("wkT", [D, HD], F32R, kind="ExternalInput").ap()
    wvT = nc.dram_tensor("wvT", [D, HD], F32R, kind="ExternalInput").ap()
    woT = nc.dram_tensor("woT", [D, D], mybir.dt.bfloat16, kind="ExternalInput").ap()
    maskTd = nc.dram_tensor("maskTd", [128, 128], F32, kind="ExternalInput").ap()
    mstepA = nc.dram_tensor(
        "mstepA", [128, 128], mybir.dt.bfloat16, kind="ExternalInput"
    ).ap()
    mstepB = nc.dram_tensor(
        "mstepB", [128, 128], mybir.dt.bfloat16, kind="ExternalInput"
    ).ap()

    y = nc.dram_tensor("y", [QT, D], F32, kind="ExternalOutput").ap()
    k_out = nc.dram_tensor("k_out", [B, S, HD], F32, kind="ExternalOutput").ap()
    v_out = nc.dram_tensor("v_out", [B, S, HD], F32, kind="ExternalOutput").ap()

    with tile.TileContext(nc, num_cores=NCORES) as tc, nc.allow_low_precision(
        reason="fp32r compute throughout"
    ):
        with (
            tc.tile_pool(name="consts", bufs=1) as consts,
            tc.tile_pool(name="qkv", bufs=1) as qkv,
            tc.tile_pool(name="dram", bufs=1, space="DRAM") as dram,
        ):
            ident_f = consts.tile([128, 128], F32)
            make_identity(nc, ident_f[:])
            ident = consts.tile([128, 128], F32R)
            nc.vector.tensor_copy(ident[:], ident_f[:])
            maskT = consts.tile([128, 128], F32)
            nc.sync.dma_start(out=maskT[:], in_=maskTd)
            mstepA_sb = consts.tile([128, 128], mybir.dt.bfloat16)
            nc.sync.dma_start(out=mstepA_sb[:], in_=mstepA)
            mstepB_sb = consts.tile([128, 128], mybir.dt.bfloat16)
            nc.sync.dma_start(out=mstepB_sb[:], in_=mstepB)
            ones_f = consts.tile([128, 64], F32)
            nc.vector.memset(ones_f[:], 1.0)
            ones_r = consts.tile([128, 64], F32R)
            nc.vector.tensor_copy(ones_r[:], ones_f[:])

            qT = [
                [qkv.tile([128, S], F32R, name=f"qT_b{b}_h{h}") for h in range(QH)]
                for b in range(B)
            ]
            kT = [qkv.tile([128, S], F32R, name=f"kT_b{b}") for b in range(B)]
            vaug = qkv.tile([128, B * NKEYT * 65], F32R)

            ag_in = dram.tile([B, QH * HD, S], mybir.dt.bfloat16)
            ag_out = dram.tile([B, NCORES * QH * HD, S], mybir.dt.bfloat16)

            with (
                tc.tile_pool(name="wpool", bufs=1) as wpool,
                tc.tile_pool(name="xpool", bufs=2) as xpool,
                tc.tile_pool(name="stageA", bufs=3) as stageA,
                tc.tile_pool(name="stageB", bufs=3) as stageB,
                tc.tile_pool(name="attnp", bufs=3) as attnp,
                tc.tile_pool(name="kcache", bufs=1) as kcache,
                tc.tile_pool(name="psA", bufs=2, space="PSUM") as psA,
                tc.tile_pool(name="psP", bufs=2, space="PSUM") as psP,
                tc.tile_pool(name="psO", bufs=2, space="PSUM") as psO,
            ):
                k_sb = kcache.tile([128, B * NKEYT * 64], F32)

                wk_sb = wpool.tile([128, KT * HD], F32R)
                nc.sync.dma_start(
                    out=wk_sb[:].rearrange("p (kt m) -> p kt m", kt=KT),
                    in_=wkT.rearrange("(kt p) m -> p kt m", p=128),
                )
                wv_sb = wpool.tile([128, KT * HD], F32R)
                nc.sync.dma_start(
                    out=wv_sb[:].rearrange("p (kt m) -> p kt m", kt=KT),
                    in_=wvT.rearrange("(kt p) m -> p kt m", p=128),
                )
                wq_sb = wpool.tile([128, KT * QH * HD], F32R)
                for qa in range(4):
                    nc.sync.dma_start(
                        out=wq_sb[:].rearrange("p (kt m) -> p kt m", kt=KT)[
                            :, qa * 4 : qa * 4 + 4
                        ],
                        in_=wqT.rearrange("(kt p) m -> p kt m", p=128)[
                            :, qa * 4 : qa * 4 + 4
                        ],
                    )

                # ---------- A-unit generator: yields after each subunit ----------
                def gen_A():
                    for b in range(B):
                        for rt in range(NRT):
                            r0 = rt * RT
                            x_t = xpool.tile([128, KT * RT], F32R, name="x_t")
                            for qa in range(4):
                                nc.scalar.dma_start(
                                    out=x_t[:].rearrange(
                                        "p (kt r) -> p kt r", kt=KT
                                    )[:, qa * 4 : qa * 4 + 4],
                                    in_=xT[b].rearrange(
                                        "(kt p) r -> p kt r", p=128
                                    )[:, qa * 4 : qa * 4 + 4, r0 : r0 + RT],
                                )
                            ps_k = psA.tile([64, RT], F32, name="ps_k", tag="psA")
                            for kt in range(KT):
                                nc.tensor.matmul(
                                    ps_k[:],
                                    wk_sb[:, kt * HD : (kt + 1) * HD],
                                    x_t[:, kt * RT : (kt + 1) * RT],
                                    start=(kt == 0),
                                    stop=(kt == KT - 1),
                                )
                            nc.vector.tensor_copy(kT[b][0:64, r0 : r0 + RT], ps_k[:])
                            yield
                            for m in range(2):
                                ps_q = psA.tile([128, RT], F32, name="ps_q", tag="psA")
                                for kt in range(KT):
                                    nc.tensor.matmul(
                                        ps_q[:],
                                        wq_sb[
                                            :,
                                            kt * 256
                                            + m * 128 : kt * 256
                                            + m * 128
                                            + 128,
                                        ],
                                        x_t[:, kt * RT : (kt + 1) * RT],
                                        start=(kt == 0),
                                        stop=(kt == KT - 1),
                                    )
                                nc.vector.tensor_copy(
                                    qT[b][2 * m][0:64, r0 : r0 + RT], ps_q[0:64, :]
                                )
                                nc.vector.tensor_copy(
                                    qT[b][2 * m + 1][64:128, r0 : r0 + RT],
                                    ps_q[64:128, :],
                                )
                                yield
                            ps_v = psA.tile([64, RT], F32, name="ps_v", tag="psA")
                            for kt in range(KT):
                                nc.tensor.matmul(
                                    ps_v[:],
                                    wv_sb[:, kt * HD : (kt + 1) * HD],
                                    x_t[:, kt * RT : (kt + 1) * RT],
                                    start=(kt == 0),
                                    stop=(kt == KT - 1),
                                )
                            vT_st = stageA.tile([64, RT], F32R, name="vT_st")
                            nc.vector.tensor_copy(vT_st[:], ps_v[:])
                            for sub in range(RT // 128):
                                keyt = (r0 + sub * 128) // 128
                                ps_t = psA.tile(
                                    [128, 64], F32R, name="ps_t", tag="psA"
                                )
                                nc.tensor.matmul(
                                    ps_t[:],
                                    vT_st[:, sub * 128 : (sub + 1) * 128],
                                    ident[0:64, 0:64],
                                    is_transpose=True,
                                )
                                col = (b * NKEYT + keyt) * 65
                                nc.vector.tensor_copy(
                                    vaug[:, col : col + 64], ps_t[:]
                                )
                                nc.vector.tensor_copy(
                                    vaug[:, col + 64 : col + 65], ones_r[:, 0:1]
                                )
                            if rt % 2 == 1:
                                # pair complete: dup kv/q dims to other half
                                p0 = (rt - 1) * RT
                                nc.sync.dma_start(
                                    out=kT[b][64:128, p0 : p0 + 2 * RT],
                                    in_=kT[b][0:64, p0 : p0 + 2 * RT],
                                )
                                for h in range(QH):
                                    if h % 2 == 0:
                                        nc.sync.dma_start(
                                            out=qT[b][h][64:128, p0 : p0 + 2 * RT],
                                            in_=qT[b][h][0:64, p0 : p0 + 2 * RT],
                                        )
                                    else:
                                        nc.sync.dma_start(
                                            out=qT[b][h][0:64, p0 : p0 + 2 * RT],
                                            in_=qT[b][h][64:128, p0 : p0 + 2 * RT],
                                        )
                            yield
                    # tail filler: k-cache transposes
                    for b in range(B):
                        for pair in range(NKEYT // 4):
                            for sub in range(4):
                                keyt = pair * 4 + sub
                                ps_t2 = psA.tile(
                                    [128, 64], F32R, name="ps_t2", tag="psA"
                                )
                                nc.tensor.matmul(
                                    ps_t2[:],
                                    kT[b][0:64, keyt * 128 : (keyt + 1) * 128],
                                    ident[0:64, 0:64],
                                    is_transpose=True,
                                )
                                colk = (b * NKEYT + keyt) * 64
                                nc.vector.tensor_copy(
                                    k_sb[:, colk : colk + 64], ps_t2[:]
                                )
                            yield
                    while True:
                        yield  # exhausted: no-op filler

                gA = gen_A()

                def take_A(n):
                    for _ in range(n):
                        next(gA)

                # ---------- B stage emitter ----------
                def emit_B_stage(b, j, n_fill):
                    nond = list(range(4 * j))
                    groups = [
                        [(nond[i], 0, QT), (nond[i + 1], 0, QT)]
                        for i in range(0, len(nond), 2)
                    ]
                    d0 = 4 * j
                    groups.append([(d0, 0, 512), (d0 + 1, 128, 384)])
                    groups.append([(d0 + 2, 256, 256), (d0 + 3, 384, 128)])
                    last_kt = 4 * j + 3
                    total_groups = len(groups) * QH
                    gi = 0
                    filled = 0
                    for h in range(QH):
                        ps_o = psO.tile([65, QT], F32, name="ps_o", tag="psO")
                        for grp in groups:
                            width = sum(g[2] for g in grp)
                            ps_p = psP.tile([128, 1024], F32, name="ps_p", tag="psP")
                            col = 0
                            for pos, (kt, qoff, n_q) in enumerate(grp):
                                p0 = 64 * (pos % 2)
                                diag = kt >= d0
                                if diag:
                                    # prefill the causal corner mask into PSUM
                                    # (bank-clearing start=True), then let the
                                    # scores matmul accumulate on top.
                                    nc.tensor.matmul(
                                        ps_p[:, col : col + 128],
                                        mstepA_sb[:],
                                        mstepB_sb[:],
                                    )
                                nc.tensor.matmul(
                                    ps_p[:, col : col + n_q],
                                    kT[b][p0 : p0 + 64, kt * 128 : (kt + 1) * 128],
                                    qT[b][h][
                                        p0 : p0 + 64, j * QT + qoff : (j + 1) * QT
                                    ],
                                    start=not diag,
                                    stop=True,
                                    skip_group_check=True,
                                )
                                col += n_q
                            at = attnp.tile([128, 1024], F32R, name="at")
                            nc.scalar.activation(
                                at[:, 0:width], ps_p[:, 0:width], EXP, scale=0.125
                            )
                            col = 0
                            for kt, qoff, n_q in grp:
                                vcol = (b * NKEYT + kt) * 65
                                nc.tensor.matmul(
                                    ps_o[:, qoff:QT],
                                    vaug[:, vcol : vcol + 65],
                                    at[:, col : col + n_q],
                                    start=(kt == 0),
                                    stop=(kt == last_kt),
                                )
                                col += n_q
                            gi += 1
                            want = gi * n_fill // total_groups
                            # clump fills into bursts of >=3 subunits so the
                            # PE gets ~5us dense stretches (HAM warm-up).
                            if want - filled >= 3 or (want > filled and gi == total_groups):
                                take_A(want - filled)
                                filled = want
                        recip = stageB.tile([65, QT], F32R, name="recip")
                        nc.vector.reciprocal(recip[64:65, :], ps_o[64:65, :])
                        bc_ps = psA.tile([64, QT], F32, name="bc_ps", tag="psA")
                        nc.tensor.matmul(bc_ps[:], ones_r[64:65, :], recip[64:65, :])
                        bc_sb = stageB.tile([64, QT], F32, name="bc_sb")
                        nc.scalar.activation(bc_sb[:], bc_ps[:], COPY)
                        outT_t = stageB.tile([64, QT], mybir.dt.bfloat16, name="outT_t")
                        nc.vector.tensor_mul(outT_t[:], ps_o[0:64, :], bc_sb[:])
                        nc.sync.dma_start(
                            out=ag_in[b][
                                h * HD : (h + 1) * HD, j * QT : (j + 1) * QT
                            ],
                            in_=outT_t[:],
                        )
                    if filled < n_fill:
                        take_A(n_fill - filled)

                # ---------- pipeline ----------
                take_A(8)  # PRE: b0 rt0-1
                for b in range(B):
                    for j in range(NQT):
                        emit_B_stage(b, j, 8)
                        if b == 0 and j == 2:
                            nc.gpsimd.dma_start(
                                out=v_out[0].rearrange("(kt p) d -> p kt d", p=128),
                                in_=vaug[:, 0 : NKEYT * 65]
                                .rearrange("p (kt e) -> p kt e", e=65)[:, :, 0:64],
                            )
                    if collective:
                        nc.gpsimd.collective_compute(
                            "AllGather",
                            mybir.AluOpType.bypass,
                            replica_groups=[list(range(NCORES))],
                            ins=[ag_in[b].opt()],
                            outs=[ag_out[b].opt()],
                        )
                    else:
                        nc.sync.dma_start(
                            out=ag_out[b][0 : QH * HD, :], in_=ag_in[b]
                        )
                nc.gpsimd.dma_start(
                    out=v_out[1].rearrange("(kt p) d -> p kt d", p=128),
                    in_=vaug[:, NKEYT * 65 : 2 * NKEYT * 65]
                    .rearrange("p (kt e) -> p kt e", e=65)[:, :, 0:64],
                )
                for b in range(B):
                    nc.sync.dma_start(
                        out=k_out[b].rearrange("(kt p) d -> p kt d", p=128),
                        in_=k_sb[
                            :, b * NKEYT * 64 : (b + 1) * NKEYT * 64
                        ].rearrange("p (kt d) -> p kt d", kt=NKEYT),
                    )

            # ========== Phase C: output projection ==========
            with (
                tc.tile_pool(name="chp", bufs=1) as chp,
                tc.tile_pool(name="wop", bufs=16) as wop,
                tc.tile_pool(name="yp", bufs=2) as yp,
                tc.tile_pool(name="psY", bufs=6, space="PSUM") as psY,
            ):
                cid = nc.sync.partition_id()
                my_b = cid // 4
                my_col = (cid % 4) * QT
                ch_all = chp.tile([128, KT * QT], mybir.dt.bfloat16, name="ch_all")
                for qa in range(4):
                    nc.sync.dma_start(
                        out=ch_all[:].rearrange("p (kt c) -> p kt c", kt=KT)[
                            :, qa * 4 : qa * 4 + 4
                        ],
                        in_=ag_out.rearrange("b (kt p) c -> p kt b c", p=128)[
                            :, qa * 4 : qa * 4 + 4, bass.ds(my_b, 1), bass.ds(my_col, QT)
                        ].rearrange("p kt b c -> p kt (b c)"),
                    )
                chunks = [ch_all[:, kt2 * QT : (kt2 + 1) * QT] for kt2 in range(KT)]
                for n in range(4):
                    ps_y = [
                        psY.tile([128, 512], F32, name=f"ps_y{mm}", tag="psY")
                        for mm in range(4)
                    ]
                    for kt2 in range(KT):
                        wo_t = wop.tile([128, 512], mybir.dt.bfloat16, name="wo_t")
                        nc.sync.dma_start(
                            out=wo_t[:],
                            in_=woT[
                                kt2 * 128 : (kt2 + 1) * 128, n * 512 : (n + 1) * 512
                            ],
                        )
                        for mm in range(4):
                            nc.tensor.matmul(
                                ps_y[mm][:],
                                chunks[kt2][:, mm * 128 : (mm + 1) * 128],
                                wo_t[:],
                                start=(kt2 == 0),
                                stop=(kt2 == KT - 1),
                            )
                    for mm in range(4):
                        y_sb = yp.tile([128, 512], F32, name="y_sb")
                        nc.scalar.activation(y_sb[:], ps_y[mm][:], COPY)
                        nc.sync.dma_start(
                            out=y[mm * 128 : (mm + 1) * 128, n * 512 : (n + 1) * 512],
                            in_=y_sb[:],
                        )

    nc.compile()
    return nc


_cache = {}


def _get_nc():
    if "nc" not in _cache:
        _cache["nc"] = _build(collective=True)
    return _cache["nc"]


def _numpy_reference(x, mask, Wq, Wk, Wv, Wo):
    """Fallback for non-causal masks: straight numpy implementation."""
    b, s, _ = x.shape
    q = (x @ Wq.T).reshape(b, s, H, HD).transpose(0, 2, 1, 3)
    k = (x @ Wk.T).reshape(b, s, KVH, HD).transpose(0, 2, 1, 3)
    v = (x @ Wv.T).reshape(b, s, KVH, HD).transpose(0, 2, 1, 3)
    kr = np.repeat(k, GROUPS, axis=1)
    vr = np.repeat(v, GROUPS, axis=1)
    scores = np.einsum("bhqd,bhkd->bhqk", q, kr) / np.sqrt(np.float32(HD))
    scores = scores + mask
    scores = scores - scores.max(axis=-1, keepdims=True)
    e = np.exp(scores)
    attn = e / e.sum(axis=-1, keepdims=True)
    out = np.einsum("bhqk,bhkd->bhqd", attn, vr)
    out = out.transpose(0, 2, 1, 3).reshape(b, s, H * HD)
    return (out @ Wo.T).astype(np.float32), k, v


def kernel(x, attention_mask, Wq, Wk, Wv, Wo):
    import ml_dtypes

    x = np.ascontiguousarray(np.asarray(x, dtype=np.float32))
    mask = np.ascontiguousarray(np.asarray(attention_mask, dtype=np.float32))
    Wq = np.ascontiguousarray(np.asarray(Wq, dtype=np.float32))
    Wk = np.ascontiguousarray(np.asarray(Wk, dtype=np.float32))
    Wv = np.ascontiguousarray(np.asarray(Wv, dtype=np.float32))
    Wo = np.ascontiguousarray(np.asarray(Wo, dtype=np.float32))

    # fast path requires the standard causal additive mask: zeros on/below
    # the diagonal, <= -1e8 above it.
    m2 = mask.reshape(S, S)
    causal = bool(
        (m2[np.tril_indices(S)] == 0.0).all()
        and (m2[np.triu_indices(S, k=1)] <= -1e8).all()
    )
    if not causal:
        return _numpy_reference(x, mask, Wq, Wk, Wv, Wo)

    xT = np.ascontiguousarray(x.transpose(0, 2, 1))
    woT = np.ascontiguousarray(Wo.T).astype(ml_dtypes.bfloat16)
    maskTd = np.ascontiguousarray(m2[0:128, 0:128].T)
    # step matrices for the PSUM mask prefill:
    # mask[dk, q] = sum_i A[i, dk] * Bm[i, q]  (= -(dk-q)*1e9 above diagonal)
    ii = np.arange(128)
    mA = (ii[:, None] <= ii[None, :]).astype(ml_dtypes.bfloat16)  # [i, dk]
    mBm = np.where(ii[:, None] > ii[None, :], np.float32(-1e9), 0.0).astype(
        ml_dtypes.bfloat16
    )  # [i, q]
    in_maps = []
    for c in range(NCORES):
        in_maps.append(
            {
                "xT": xT,
                "wqT": np.ascontiguousarray(Wq[256 * c : 256 * (c + 1), :].T),
                "wkT": np.ascontiguousarray(Wk[64 * c : 64 * (c + 1), :].T),
                "wvT": np.ascontiguousarray(Wv[64 * c : 64 * (c + 1), :].T),
                "woT": woT,
                "maskTd": maskTd,
                "mstepA": mA,
                "mstepB": mBm,
            }
        )

    res = bass_utils.run_bass_kernel_spmd(
        _get_nc(), in_maps, core_ids=list(range(NCORES))
    )

    out = np.empty((B, S, D), np.float32)
    k = np.empty((B, KVH, S, HD), np.float32)
    v = np.empty((B, KVH, S, HD), np.float32)
    for c in range(NCORES):
        r = res.results[c]
        out[c // 4, 512 * (c % 4) : 512 * (c % 4) + 512, :] = r["y"]
        k[:, c, :, :] = r["k_out"]
        v[:, c, :, :] = r["v_out"]
    return out, k, v
